# revision 24
# baseline (speedup 1.0000x reference)
"""Trainium2 Bass kernel for nn_Attention_90220083019846.

Multi-head attention block: q/k/v = X@W{q,k,v}, scores = (q+cb)@k^T,
softmax, O = P@v, Z = X + O@Wo^T + b, LayerNorm.

Sharding over 8 NeuronCores: data-parallel over batch (2 groups of 4 cores) x
tensor-parallel over heads (4 heads per core). Output projection partial sums
are combined with a ReduceScatter within each batch group; residual +
LayerNorm run on the scattered shards.

v2 rework, driven by the p-state ramp model (a stalled PE restarts at half
clock for 3us): the whole PE stream is kept gapless. Score PSUM rotates
through three 2-bank slots so the scheduler can run scores two groups ahead
of the exp; the P@v accumulator has two banks so the next head starts while
the previous head's Oh is evacuated. Head pairs are packed: the odd head's v
columns sit at 64..127 (ones column at 63), so Oh lands on partitions 64..127
and the output projection contracts a full K=128 with no zero padding,
halving its matmul count. The softmax-sums broadcast rides a full-rate f32r
matmul whose slot reuses the score pool. LayerNorm residuals (with Wo_b
pre-added on host) are preloaded to SBUF and the LN blocks are slotted into
the DVE stream behind each ReduceScatter via schedule hints; the last query
block reduce-scatters in 128-row chunks to shrink the tail.
"""

import contextlib
import ctypes
import sys
import types

sys.path.insert(0, "/opt/trn_rl_repo")

import numpy as np

# ---------------------------------------------------------------- profile hook
# The agent image's antenv lacks axon_hooks; provide it so that
# run_bass_kernel_spmd(trace=True) / BASS_TRACE=1 can capture NTFF profiles.
def _install_profile_hook():
    if "antenv.axon_hooks" in sys.modules:
        return
    try:
        import antenv
    except ImportError:
        return
    mod = types.ModuleType("antenv.axon_hooks")
    mod._hook = None
    mod.set_axon_ntff_profile_hook = lambda h: setattr(mod, "_hook", h)
    mod.get_axon_ntff_profile_hook = lambda: mod._hook
    sys.modules["antenv.axon_hooks"] = mod
    antenv.axon_hooks = mod
    try:
        lib = ctypes.CDLL("/opt/axon/libaxon_pjrt.so")
        if not hasattr(lib, "axon_start_nrt_profile"):
            return
        lib.axon_start_nrt_profile.argtypes = [
            ctypes.POINTER(ctypes.c_int64),
            ctypes.c_size_t,
        ]
        lib.axon_start_nrt_profile.restype = ctypes.c_int64
        lib.axon_stop_nrt_profile.argtypes = [ctypes.c_char_p]
        lib.axon_stop_nrt_profile.restype = ctypes.c_int64

        @contextlib.contextmanager
        def _hook(output_dir, device_ids):
            import jax

            jax.devices()
            if device_ids:
                ids = (ctypes.c_int64 * len(device_ids))(*device_ids)
                rc = lib.axon_start_nrt_profile(ids, len(device_ids))
            else:
                rc = lib.axon_start_nrt_profile(None, 0)
            if rc != 0:
                raise RuntimeError(f"axon_start_nrt_profile rc={rc}")
            try:
                yield
            finally:
                n = lib.axon_stop_nrt_profile(str(output_dir).encode())
                print(f"profile: {n} file(s) written to {output_dir}", file=sys.stderr)

        mod.set_axon_ntff_profile_hook(_hook)
    except OSError:
        pass


_install_profile_hook()

# ------------------------------------------------------------------- constants
B, L, D, H, HD = 2, 2048, 1024, 16, 64
NCORES = 8
GROUP = 4            # cores per batch group (tensor-parallel over heads)
HL = H // GROUP      # local heads per core
NP = HL // 2         # local head pairs
DL = HL * HD         # local head dims per core
QB = 512             # query block (tokens per pipeline chunk)
NQB = L // QB
NKC = L // 128       # key chunks
RG = [[0, 1, 2, 3], [4, 5, 6, 7]]
LN_EPS = 1e-5
RSQRT_MAGIC = 0x5F3759DF
# schedule hints (ms of estimated time) for when each LN block's DVE work may
# be slotted in; biased late so a tardy ReduceScatter never blocks the
# in-order DVE queue ahead of attention work
LN_HINTS = [0.5, 0.51, 0.52, 0.53]

_PROGRAM = None
LAST_RESULT = None


def _build_program():
    import concourse.tile as tile
    from concourse import bacc, mybir

    fr = mybir.dt.float32r
    f32 = mybir.dt.float32
    bf16 = mybir.dt.bfloat16
    i32 = mybir.dt.int32
    Exp = mybir.ActivationFunctionType.Exp
    Alu = mybir.AluOpType

    nc = bacc.Bacc("TRN2", target_bir_lowering=False, debug=False,
                   num_devices=NCORES)

    xt_d = nc.dram_tensor("xt", (D, L), bf16, kind="ExternalInput").ap()
    wq_d = nc.dram_tensor("wq", (D, DL), bf16, kind="ExternalInput").ap()
    wk_d = nc.dram_tensor("wk", (D, DL), bf16, kind="ExternalInput").ap()
    wv_d = nc.dram_tensor("wv", (D, DL), bf16, kind="ExternalInput").ap()
    wot_d = nc.dram_tensor("wot", (DL, D), bf16, kind="ExternalInput").ap()
    cb_d = nc.dram_tensor("cb", (DL, 1), f32, kind="ExternalInput").ap()
    xres_d = nc.dram_tensor("xres", (QB, D), f32, kind="ExternalInput").ap()
    lng_d = nc.dram_tensor("lng", (1, D), f32, kind="ExternalInput").ap()
    lnb_d = nc.dram_tensor("lnb", (1, D), f32, kind="ExternalInput").ap()
    out_d = nc.dram_tensor("out", (QB, D), f32, kind="ExternalOutput").ap()

    ccin = [nc.dram_tensor(f"ccin{t}", (QB, D), bf16, kind="Internal").ap()
            for t in range(NQB)]
    ccout = [nc.dram_tensor(f"ccout{t}", (QB // GROUP, D), bf16,
                            kind="Internal").ap()
             for t in range(NQB)]

    with tile.TileContext(nc) as tc, contextlib.ExitStack() as ctx:
        # ---------------- persistent pools
        wp = ctx.enter_context(tc.tile_pool(name="wp", bufs=1))
        kqv = ctx.enter_context(tc.tile_pool(name="kqv", bufs=1))
        cons = ctx.enter_context(tc.tile_pool(name="cons", bufs=1))
        # PSUM: 3 score slots x 2 banks + 2 P@v accumulator banks = 8 banks.
        # The sums-broadcast (rb) and output-projection (zp) tiles rotate
        # through the score pool.
        psp = ctx.enter_context(tc.tile_pool(name="psp", bufs=3, space="PSUM"))
        ohp = ctx.enter_context(tc.tile_pool(name="ohp", bufs=2, space="PSUM"))

        wq_t = wp.tile([128, 8, DL], bf16)
        wk_t = wp.tile([128, 8, DL], bf16)
        wv_t = wp.tile([128, 8, DL], bf16)
        wot_t = wp.tile([128, NP, D], bf16)
        # wk lands first, split per contraction chunk so the very first
        # projection matmul only waits on chunk 0
        wk_r = wk_d.rearrange("(c p) o -> p c o", p=128)
        for c in range(8):
            nc.sync.dma_start(out=wk_t[:, c, :], in_=wk_r[:, c, :])

        # k^T with the other head of the pair zeroed (full-square lhsT);
        # q^T keeps both heads (zero weights ignore the other head's rows)
        kt_ev = kqv.tile([128, 2, L], bf16)
        kt_od = kqv.tile([128, 2, L], bf16)
        qt = kqv.tile([128, 2, L], bf16)     # q^T (+cb)
        # v | ones columns; per head pair: even head dims at block cols 0..63
        # of its 128-block with ones at 64, odd head dims at 64..127 of its
        # block with ones at 0
        vaug = kqv.tile([128, NKC, HL * 128], bf16)
        ohn = kqv.tile([128, NP, L], bf16)   # normalized Oh^T, head pairs packed
        xres_t = kqv.tile([128, 4, D], f32)  # residual rows (+Wo_b), preloaded

        cb_t = cons.tile([128, 2], f32)
        nc.sync.dma_start(out=cb_t, in_=cb_d.rearrange("(m p) x -> p (m x)", p=128))
        # lhsT for the K=1 sums-broadcast outer product (any single row).
        # bf16: the f32r matmul path only yields valid output on the first
        # 64 partitions, bf16 covers all 128.
        ones_t = cons.tile([128, 128], bf16)
        nc.vector.memset(ones_t, 1.0)
        lng_t = cons.tile([128, D], f32)
        lnb_t = cons.tile([128, D], f32)
        magic_t = cons.tile([128, 1], i32)
        nc.vector.memset(magic_t, RSQRT_MAGIC)
        # row masks: keep one head of a pair, zero the other (f32 scalars)
        mask_lo = cons.tile([128, 1], f32)
        mask_hi = cons.tile([128, 1], f32)
        nc.vector.memset(mask_lo, 0.0)
        nc.vector.memset(mask_lo[0:64, :], 1.0)
        nc.vector.memset(mask_hi, 0.0)
        nc.vector.memset(mask_hi[64:128, :], 1.0)

        # vaug constant columns: ones at even-block col 64 / odd-block col 0
        # (pair-relative 128), zeros on the unused strips (the zeros feed
        # matmul contractions, so they must be real zeros, not garbage)
        vv = vaug.rearrange("p k (pair x) -> p k pair x", pair=NP)
        nc.vector.memset(vv[:, :, :, 64:65], 1.0)
        nc.vector.memset(vv[:, :, :, 128:129], 1.0)
        nc.vector.memset(vv[:, :, :, 65:128], 0.0)
        nc.vector.memset(vv[:, :, :, 129:192], 0.0)

        # ---------------- stage A: projections (needs X^T)
        with tc.tile_pool(name="xtp", bufs=1) as xtp:
            xt = xtp.tile([128, 8, L], bf16)
            for t4 in range(NQB):
                for c in range(8):
                    nc.sync.dma_start(
                        out=xt[:, c, QB * t4:QB * (t4 + 1)],
                        in_=xt_d[128 * c:128 * (c + 1), QB * t4:QB * (t4 + 1)])
                if t4 == 0:
                    nc.sync.dma_start(
                        out=wv_t, in_=wv_d.rearrange("(c p) o -> p c o", p=128))
                    nc.sync.dma_start(
                        out=wq_t, in_=wq_d.rearrange("(c p) o -> p c o", p=128))
                    nc.sync.dma_start(
                        out=wot_t,
                        in_=wot_d.rearrange("(h p) e -> p h e", p=128))
            nc.sync.dma_start(
                out=xres_t, in_=xres_d.rearrange("(q p) e -> p q e", p=128))

            # interleave k/q/v projections by t4 so compute starts as soon as
            # the first X chunk lands
            for t4 in range(NQB):
                tsl = slice(QB * t4, QB * (t4 + 1))
                for w_t, is_q in ((wk_t, False), (wq_t, True)):
                    for m in range(2):
                        ps = psp.tile([128, 1024], f32, tag="ps")
                        for c in range(8):
                            nc.tensor.matmul(
                                out=ps[:, 0:512],
                                lhsT=w_t[:, c, 128 * m:128 * (m + 1)],
                                rhs=xt[:, c, tsl],
                                start=(c == 0), stop=(c == 7),
                            )
                        if is_q:
                            # ACT is idle until the first exp; offload the
                            # bias-add evacuation there
                            nc.scalar.add(out=qt[:, m, tsl],
                                          in_=ps[:, 0:512],
                                          add=cb_t[:, m:m + 1])
                        else:
                            nc.vector.tensor_scalar_mul(
                                out=kt_ev[:, m, tsl], in0=ps[:, 0:512],
                                scalar1=mask_lo)
                            nc.scalar.activation(
                                out=kt_od[:, m, tsl], in_=ps[:, 0:512],
                                func=mybir.ActivationFunctionType.Identity,
                                bias=0.0, scale=mask_hi)
                # v: tokens on partitions, head dims free
                for kc in range(4 * t4, 4 * (t4 + 1)):
                    ps = psp.tile([128, 1024], f32, tag="ps")
                    for c in range(8):
                        nc.tensor.matmul(
                            out=ps[:, 0:DL],
                            lhsT=xt[:, c, 128 * kc:128 * (kc + 1)],
                            rhs=wv_t[:, c, :],
                            start=(c == 0), stop=(c == 7),
                        )
                    pv = ps[:, 0:DL].rearrange("p (pair x) -> p pair x", pair=NP)
                    nc.vector.tensor_copy(out=vv[:, kc, :, 0:64],
                                          in_=pv[:, :, 0:64])
                    nc.vector.tensor_copy(out=vv[:, kc, :, 192:256],
                                          in_=pv[:, :, 64:128])

        # LN constants are not needed until late; keep their DMAs off the
        # startup critical path
        nc.sync.dma_start(out=lng_t, in_=lng_d.partition_broadcast(128))
        nc.sync.dma_start(out=lnb_t, in_=lnb_d.partition_broadcast(128))

        # ---------------- stage B (attention) + stage C (proj/RS/LN)
        with tc.tile_pool(name="ptp", bufs=4) as ptp, \
             tc.tile_pool(name="ohsp", bufs=2) as ohsp, \
             tc.tile_pool(name="recp", bufs=2) as recp, \
             tc.tile_pool(name="zevp", bufs=2) as zevp, \
             tc.tile_pool(name="lnp", bufs=2) as lnp:

            def norm_evac(oh):
                """Evacuate Oh+sums to SBUF right behind the last P@v."""
                ohs = ohsp.tile([128, 512], bf16, tag="ohs")
                nc.vector.tensor_copy(out=ohs, in_=oh)
                return ohs

            def norm_finish(qb, h, ohs):
                """Broadcast softmax sums, normalize into ohn. Deferred one
                head so the rb matmul never head-blocks the PE queue on the
                DVE evacuation."""
                pair = h // 2
                odd = h % 2
                rb = psp.tile([128, 1024], f32, tag="ps")
                rec = recp.tile([128, 512], f32, tag="rec")
                tsl = slice(QB * qb, QB * (qb + 1))
                # sums sit on row 64 (even head) / row 0 (odd head); the K=1
                # matmul outer-products that row across all 128 partitions
                srow = 0 if odd else 64
                dst = slice(64, 128) if odd else slice(0, 64)
                nc.tensor.matmul(out=rb[:, 0:512],
                                 lhsT=ones_t[srow:srow + 1, :],
                                 rhs=ohs[srow:srow + 1, :],
                                 start=True, stop=True)
                # full-tile: the custom-DVE reciprocal misbehaves at a
                # non-zero base partition; rb is identical on every row
                nc.vector.reciprocal_approx_fast(out=rec, in_=rb[:, 0:512])
                nc.vector.tensor_mul(out=ohn[dst, pair, tsl],
                                     in0=ohs[dst, :], in1=rec[dst, :])

            def oproj_tcl(qb, tcl):
                """One 128-token output-projection chunk; RS after the last."""
                t0 = QB * qb + 128 * tcl
                zp = psp.tile([128, 1024], f32, tag="ps")
                for ec in range(2):
                    for pair in range(NP):
                        nc.tensor.matmul(
                            out=zp[:, 512 * ec:512 * (ec + 1)],
                            lhsT=ohn[:, pair, t0:t0 + 128],
                            rhs=wot_t[:, pair, 512 * ec:512 * (ec + 1)],
                            start=(pair == 0), stop=(pair == NP - 1),
                        )
                zev = zevp.tile([128, D], bf16)
                nc.vector.tensor_copy(out=zev, in_=zp)
                nc.sync.dma_start(
                    out=ccin[qb][128 * tcl:128 * (tcl + 1), :], in_=zev)
                if tcl == QB // 128 - 1:
                    nc.gpsimd.collective_compute(
                        "ReduceScatter", Alu.add,
                        ins=[ccin[qb][:]], outs=[ccout[qb][:]],
                        replica_groups=RG,
                    )

            def layernorm(qb):
                """Residual + LayerNorm on this core's scattered shard."""
                ccz = lnp.tile([128, D], bf16, tag="ccz")
                nc.sync.dma_start(out=ccz, in_=ccout[qb])
                zt = lnp.tile([128, D], f32, tag="zt")
                nc.vector.tensor_copy(out=zt, in_=ccz)
                nc.vector.tensor_add(out=zt, in0=zt, in1=xres_t[:, qb, :])

                stats = lnp.tile([128, 2, 6], f32, tag="stats")
                for sg in range(2):
                    nc.vector.bn_stats(out=stats[:, sg, :],
                                       in_=zt[:, 512 * sg:512 * (sg + 1)])
                mv = lnp.tile([128, 2], f32, tag="mv")
                nc.vector.bn_aggr(out=mv, in_=stats)

                # rstd = rsqrt(var + eps), DVE-only (avoids ACT table thrash)
                ve = lnp.tile([128, 1], f32, tag="ve")
                nc.vector.tensor_scalar_add(out=ve, in0=mv[:, 1:2], scalar1=LN_EPS)
                y = lnp.tile([128, 1], f32, tag="y")
                nc.vector.tensor_scalar(
                    out=y.bitcast(i32), in0=ve.bitcast(i32), scalar1=1,
                    scalar2=None, op0=Alu.logical_shift_right)
                nc.vector.tensor_sub(out=y.bitcast(i32), in0=magic_t,
                                     in1=y.bitcast(i32))
                tnw = lnp.tile([128, 1], f32, tag="tnw")
                for _ in range(2):
                    nc.vector.tensor_mul(out=tnw, in0=ve, in1=y)
                    nc.vector.tensor_mul(out=tnw, in0=tnw, in1=y)
                    nc.vector.tensor_scalar(out=tnw, in0=tnw, scalar1=-0.5,
                                            scalar2=1.5, op0=Alu.mult, op1=Alu.add)
                    nc.vector.tensor_mul(out=y, in0=y, in1=tnw)

                nc.vector.tensor_scalar(out=zt, in0=zt, scalar1=mv[:, 0:1],
                                        scalar2=y, op0=Alu.subtract, op1=Alu.mult)
                nc.vector.tensor_mul(out=zt, in0=zt, in1=lng_t)
                nc.vector.tensor_add(out=zt, in0=zt, in1=lnb_t)
                nc.sync.dma_start(out=out_d[128 * qb:128 * (qb + 1), :], in_=zt)

            pending = None
            for qb in range(NQB):  # noqa: B007

                for h in range(HL):
                    mi = h // 2
                    ktp = kt_ev if h % 2 == 0 else kt_od
                    qT_b = qt[:, mi, QB * qb:QB * (qb + 1)]
                    oh = ohp.tile([128, 512], f32, tag="oh")
                    for g in range(8):
                        # spread the previous block's output projection over
                        # this head's score groups so its PSUM slots never
                        # bunch up in the rotation
                        if qb > 0 and h == 1 and g >= 4:
                            oproj_tcl(qb - 1, g - 4)
                        st = psp.tile([128, 1024], f32, tag="ps")
                        for i in range(2):
                            kc = 2 * g + i
                            nc.tensor.matmul(
                                out=st[:, 512 * i:512 * (i + 1)],
                                lhsT=ktp[:, mi, 128 * kc:128 * (kc + 1)],
                                rhs=qT_b,
                                start=True, stop=True,
                            )
                        pt = ptp.tile([128, 1024], bf16, tag="pt")
                        nc.scalar.activation(out=pt, in_=st, func=Exp)
                        for i in range(2):
                            kc = 2 * g + i
                            nc.tensor.matmul(
                                out=oh,
                                lhsT=vaug[:, kc, 128 * h:128 * (h + 1)],
                                rhs=pt[:, 512 * i:512 * (i + 1)],
                                start=(kc == 0), stop=(kc == NKC - 1),
                            )
                    ohs = norm_evac(oh)
                    if pending is not None:
                        norm_finish(*pending)
                    pending = (qb, h, ohs)
                if qb == NQB - 1:
                    norm_finish(*pending)
                    pending = None
                    for tcl in range(QB // 128):
                        oproj_tcl(qb, tcl)

            # LN blocks run after the whole attention DVE stream: their data
            # (ReduceScatter outputs) is long ready by then for qb 0..2, and
            # the DVE work overlaps the final ReduceScatter waits, so the
            # in-order DVE queue can never block attention work.
            for qb in range(NQB):
                with tc.tile_wait_until(LN_HINTS[qb]):
                    layernorm(qb)

    nc.compile()
    return nc


def _get_program():
    global _PROGRAM
    if _PROGRAM is None:
        _PROGRAM = _build_program()
    return _PROGRAM


def kernel(X, Y, Wq, Wk, Wv, cb, Wo_w, Wo_b, ln_g, ln_b):
    import ml_dtypes
    from concourse import bass_utils

    prog = _get_program()
    bf = ml_dtypes.bfloat16

    X = np.asarray(X, dtype=np.float32)
    Wq = np.asarray(Wq, dtype=np.float32)
    Wk = np.asarray(Wk, dtype=np.float32)
    Wv = np.asarray(Wv, dtype=np.float32)
    cb = np.asarray(cb, dtype=np.float32)
    Wo_w = np.asarray(Wo_w, dtype=np.float32)
    Wo_b = np.asarray(Wo_b, dtype=np.float32)
    ln_g = np.asarray(ln_g, dtype=np.float32)
    ln_b = np.asarray(ln_b, dtype=np.float32)

    WoT = np.ascontiguousarray(Wo_w.T)

    def core_rows(r):
        """Global row indices (within a batch) this core outputs, in order."""
        return np.concatenate(
            [np.arange(QB * t + 128 * r, QB * t + 128 * r + 128)
             for t in range(NQB)])

    in_maps = []
    for c in range(NCORES):
        b, hp, r = c // GROUP, c % GROUP, c % GROUP
        Xb = X[b]
        rows = core_rows(r)
        csl = slice(DL * hp, DL * (hp + 1))
        in_maps.append({
            "xt": np.ascontiguousarray(Xb.T).astype(bf),
            "xres": np.ascontiguousarray(Xb[rows] + Wo_b),
            "wq": np.ascontiguousarray(Wq[:, csl]).astype(bf),
            "wk": np.ascontiguousarray(Wk[:, csl]).astype(bf),
            "wv": np.ascontiguousarray(Wv[:, csl]).astype(bf),
            "wot": np.ascontiguousarray(WoT[csl, :]).astype(bf),
            "cb": np.ascontiguousarray(cb[csl].reshape(DL, 1)),
            "lng": np.ascontiguousarray(ln_g.reshape(1, D)),
            "lnb": np.ascontiguousarray(ln_b.reshape(1, D)),
        })

    res = bass_utils.run_bass_kernel_spmd(prog, in_maps, core_ids=list(range(NCORES)))
    global LAST_RESULT
    LAST_RESULT = res

    out = np.empty((B, L, D), np.float32)
    for cid in range(NCORES):
        b, r = cid // GROUP, cid % GROUP
        o = res.results[cid]["out"]
        out[b, core_rows(r)] = o
    return out


if __name__ == "__main__":
    rng = np.random.default_rng(0)
    ins = {
        "X": rng.standard_normal((B, L, D)).astype(np.float32),
        "Y": rng.standard_normal((B, L, D)).astype(np.float32),
        "Wq": (rng.uniform(-1, 1, (D, D)) / 32).astype(np.float32),
        "Wk": (rng.uniform(-1, 1, (D, D)) / 32).astype(np.float32),
        "Wv": (rng.uniform(-1, 1, (D, D)) / 32).astype(np.float32),
        "cb": np.zeros(D, np.float32),
        "Wo_w": (rng.uniform(-1, 1, (D, D)) / 32).astype(np.float32),
        "Wo_b": (rng.uniform(-1, 1, D) / 32).astype(np.float32),
        "ln_g": np.ones(D, np.float32),
        "ln_b": np.zeros(D, np.float32),
    }
    out = kernel(**ins)
    print("out", out.shape, out.dtype, float(np.abs(out).max()))
    print("exec_time_ns:", LAST_RESULT.exec_time_ns)


# revision 25
# speedup vs baseline: 1.0158x; 1.0158x over previous
"""Trainium2 Bass kernel for nn_Attention_90220083019846.

Multi-head attention block: q/k/v = X@W{q,k,v}, scores = (q+cb)@k^T,
softmax, O = P@v, Z = X + O@Wo^T + b, LayerNorm.

Sharding over 8 NeuronCores: data-parallel over batch (2 groups of 4 cores) x
tensor-parallel over heads (4 heads per core). Output projection partial sums
are combined with a ReduceScatter within each batch group; residual +
LayerNorm run on the scattered shards.

v2 rework, driven by the p-state ramp model (a stalled PE restarts at half
clock for 3us): the whole PE stream is kept gapless. Score PSUM rotates
through three 2-bank slots so the scheduler can run scores two groups ahead
of the exp; the P@v accumulator has two banks so the next head starts while
the previous head's Oh is evacuated. Head pairs are packed: the odd head's v
columns sit at 64..127 (ones column at 63), so Oh lands on partitions 64..127
and the output projection contracts a full K=128 with no zero padding,
halving its matmul count. The softmax-sums broadcast rides a full-rate f32r
matmul whose slot reuses the score pool. LayerNorm residuals (with Wo_b
pre-added on host) are preloaded to SBUF and the LN blocks are slotted into
the DVE stream behind each ReduceScatter via schedule hints; the last query
block reduce-scatters in 128-row chunks to shrink the tail.
"""

import contextlib
import ctypes
import sys
import types

sys.path.insert(0, "/opt/trn_rl_repo")

import numpy as np

# ---------------------------------------------------------------- profile hook
# The agent image's antenv lacks axon_hooks; provide it so that
# run_bass_kernel_spmd(trace=True) / BASS_TRACE=1 can capture NTFF profiles.
def _install_profile_hook():
    if "antenv.axon_hooks" in sys.modules:
        return
    try:
        import antenv
    except ImportError:
        return
    mod = types.ModuleType("antenv.axon_hooks")
    mod._hook = None
    mod.set_axon_ntff_profile_hook = lambda h: setattr(mod, "_hook", h)
    mod.get_axon_ntff_profile_hook = lambda: mod._hook
    sys.modules["antenv.axon_hooks"] = mod
    antenv.axon_hooks = mod
    try:
        lib = ctypes.CDLL("/opt/axon/libaxon_pjrt.so")
        if not hasattr(lib, "axon_start_nrt_profile"):
            return
        lib.axon_start_nrt_profile.argtypes = [
            ctypes.POINTER(ctypes.c_int64),
            ctypes.c_size_t,
        ]
        lib.axon_start_nrt_profile.restype = ctypes.c_int64
        lib.axon_stop_nrt_profile.argtypes = [ctypes.c_char_p]
        lib.axon_stop_nrt_profile.restype = ctypes.c_int64

        @contextlib.contextmanager
        def _hook(output_dir, device_ids):
            import jax

            jax.devices()
            if device_ids:
                ids = (ctypes.c_int64 * len(device_ids))(*device_ids)
                rc = lib.axon_start_nrt_profile(ids, len(device_ids))
            else:
                rc = lib.axon_start_nrt_profile(None, 0)
            if rc != 0:
                raise RuntimeError(f"axon_start_nrt_profile rc={rc}")
            try:
                yield
            finally:
                n = lib.axon_stop_nrt_profile(str(output_dir).encode())
                print(f"profile: {n} file(s) written to {output_dir}", file=sys.stderr)

        mod.set_axon_ntff_profile_hook(_hook)
    except OSError:
        pass


_install_profile_hook()

# ------------------------------------------------------------------- constants
B, L, D, H, HD = 2, 2048, 1024, 16, 64
NCORES = 8
GROUP = 4            # cores per batch group (tensor-parallel over heads)
HL = H // GROUP      # local heads per core
NP = HL // 2         # local head pairs
DL = HL * HD         # local head dims per core
QB = 512             # query block (tokens per pipeline chunk)
NQB = L // QB
NKC = L // 128       # key chunks
RG = [[0, 1, 2, 3], [4, 5, 6, 7]]
LN_EPS = 1e-5
RSQRT_MAGIC = 0x5F3759DF
# schedule hints (ms of estimated time) for when each LN block's DVE work may
# be slotted in; biased late so a tardy ReduceScatter never blocks the
# in-order DVE queue ahead of attention work
LN_HINTS = [0.200, 0.203, 0.206, 0.209]

_PROGRAM = None
LAST_RESULT = None


def _build_program():
    import concourse.tile as tile
    from concourse import bacc, mybir

    fr = mybir.dt.float32r
    f32 = mybir.dt.float32
    bf16 = mybir.dt.bfloat16
    i32 = mybir.dt.int32
    Exp = mybir.ActivationFunctionType.Exp
    Alu = mybir.AluOpType

    nc = bacc.Bacc("TRN2", target_bir_lowering=False, debug=False,
                   num_devices=NCORES)

    xt_d = nc.dram_tensor("xt", (D, L), bf16, kind="ExternalInput").ap()
    wq_d = nc.dram_tensor("wq", (D, DL), bf16, kind="ExternalInput").ap()
    wk_d = nc.dram_tensor("wk", (D, DL), bf16, kind="ExternalInput").ap()
    wv_d = nc.dram_tensor("wv", (D, DL), bf16, kind="ExternalInput").ap()
    wot_d = nc.dram_tensor("wot", (DL, D), bf16, kind="ExternalInput").ap()
    cb_d = nc.dram_tensor("cb", (DL, 1), f32, kind="ExternalInput").ap()
    xres_d = nc.dram_tensor("xres", (QB, D), f32, kind="ExternalInput").ap()
    lng_d = nc.dram_tensor("lng", (1, D), f32, kind="ExternalInput").ap()
    lnb_d = nc.dram_tensor("lnb", (1, D), f32, kind="ExternalInput").ap()
    out_d = nc.dram_tensor("out", (QB, D), f32, kind="ExternalOutput").ap()

    ccin = [nc.dram_tensor(f"ccin{t}", (QB, D), bf16, kind="Internal").ap()
            for t in range(NQB)]
    ccout = [nc.dram_tensor(f"ccout{t}", (QB // GROUP, D), bf16,
                            kind="Internal").ap()
             for t in range(NQB)]

    with tile.TileContext(nc) as tc, contextlib.ExitStack() as ctx:
        # ---------------- persistent pools
        wp = ctx.enter_context(tc.tile_pool(name="wp", bufs=1))
        kqv = ctx.enter_context(tc.tile_pool(name="kqv", bufs=1))
        cons = ctx.enter_context(tc.tile_pool(name="cons", bufs=1))
        # PSUM: 3 score slots x 2 banks + 2 P@v accumulator banks = 8 banks.
        # The sums-broadcast (rb) and output-projection (zp) tiles rotate
        # through the score pool.
        psp = ctx.enter_context(tc.tile_pool(name="psp", bufs=3, space="PSUM"))
        ohp = ctx.enter_context(tc.tile_pool(name="ohp", bufs=2, space="PSUM"))

        wq_t = wp.tile([128, 8, DL], bf16)
        wk_t = wp.tile([128, 8, DL], bf16)
        wv_t = wp.tile([128, 8, DL], bf16)
        wot_t = wp.tile([128, NP, D], bf16)
        # wk lands first, split per contraction chunk so the very first
        # projection matmul only waits on chunk 0
        wk_r = wk_d.rearrange("(c p) o -> p c o", p=128)
        for c in range(8):
            nc.sync.dma_start(out=wk_t[:, c, :], in_=wk_r[:, c, :])

        # k^T with the other head of the pair zeroed (full-square lhsT);
        # q^T keeps both heads (zero weights ignore the other head's rows)
        kt_ev = kqv.tile([128, 2, L], bf16)
        kt_od = kqv.tile([128, 2, L], bf16)
        qt = kqv.tile([128, 2, L], bf16)     # q^T (+cb)
        # v | ones columns; per head pair: even head dims at block cols 0..63
        # of its 128-block with ones at 64, odd head dims at 64..127 of its
        # block with ones at 0
        vaug = kqv.tile([128, NKC, HL * 128], bf16)
        ohn = kqv.tile([128, NP, L], bf16)   # normalized Oh^T, head pairs packed
        xres_t = kqv.tile([128, 4, D], f32)  # residual rows (+Wo_b), preloaded

        cb_t = cons.tile([128, 2], f32)
        nc.sync.dma_start(out=cb_t, in_=cb_d.rearrange("(m p) x -> p (m x)", p=128))
        # lhsT for the K=1 sums-broadcast outer product (any single row).
        # bf16: the f32r matmul path only yields valid output on the first
        # 64 partitions, bf16 covers all 128.
        ones_t = cons.tile([128, 128], bf16)
        nc.vector.memset(ones_t, 1.0)
        lng_t = cons.tile([128, D], f32)
        lnb_t = cons.tile([128, D], f32)
        magic_t = cons.tile([128, 1], i32)
        nc.vector.memset(magic_t, RSQRT_MAGIC)
        # row masks: keep one head of a pair, zero the other (f32 scalars)
        mask_lo = cons.tile([128, 1], f32)
        mask_hi = cons.tile([128, 1], f32)
        nc.vector.memset(mask_lo, 0.0)
        nc.vector.memset(mask_lo[0:64, :], 1.0)
        nc.vector.memset(mask_hi, 0.0)
        nc.vector.memset(mask_hi[64:128, :], 1.0)

        # vaug constant columns: ones at even-block col 64 / odd-block col 0
        # (pair-relative 128), zeros on the unused strips (the zeros feed
        # matmul contractions, so they must be real zeros, not garbage)
        vv = vaug.rearrange("p k (pair x) -> p k pair x", pair=NP)
        nc.vector.memset(vv[:, :, :, 64:65], 1.0)
        nc.vector.memset(vv[:, :, :, 128:129], 1.0)
        nc.vector.memset(vv[:, :, :, 65:128], 0.0)
        nc.vector.memset(vv[:, :, :, 129:192], 0.0)

        # ---------------- stage A: projections (needs X^T)
        with tc.tile_pool(name="xtp", bufs=1) as xtp:
            xt = xtp.tile([128, 8, L], bf16)
            for t4 in range(NQB):
                for c in range(8):
                    nc.sync.dma_start(
                        out=xt[:, c, QB * t4:QB * (t4 + 1)],
                        in_=xt_d[128 * c:128 * (c + 1), QB * t4:QB * (t4 + 1)])
                if t4 == 0:
                    nc.sync.dma_start(
                        out=wv_t, in_=wv_d.rearrange("(c p) o -> p c o", p=128))
                    nc.sync.dma_start(
                        out=wq_t, in_=wq_d.rearrange("(c p) o -> p c o", p=128))
                    nc.sync.dma_start(
                        out=wot_t,
                        in_=wot_d.rearrange("(h p) e -> p h e", p=128))
            nc.sync.dma_start(
                out=xres_t, in_=xres_d.rearrange("(q p) e -> p q e", p=128))

            # interleave k/q/v projections by t4 so compute starts as soon as
            # the first X chunk lands
            for t4 in range(NQB):
                tsl = slice(QB * t4, QB * (t4 + 1))
                for w_t, is_q in ((wk_t, False), (wq_t, True)):
                    for m in range(2):
                        ps = psp.tile([128, 1024], f32, tag="ps")
                        for c in range(8):
                            nc.tensor.matmul(
                                out=ps[:, 0:512],
                                lhsT=w_t[:, c, 128 * m:128 * (m + 1)],
                                rhs=xt[:, c, tsl],
                                start=(c == 0), stop=(c == 7),
                            )
                        if is_q:
                            # ACT is idle until the first exp; offload the
                            # bias-add evacuation there
                            nc.scalar.add(out=qt[:, m, tsl],
                                          in_=ps[:, 0:512],
                                          add=cb_t[:, m:m + 1])
                        else:
                            nc.vector.tensor_scalar_mul(
                                out=kt_ev[:, m, tsl], in0=ps[:, 0:512],
                                scalar1=mask_lo)
                            nc.scalar.activation(
                                out=kt_od[:, m, tsl], in_=ps[:, 0:512],
                                func=mybir.ActivationFunctionType.Identity,
                                bias=0.0, scale=mask_hi)
                # v: tokens on partitions, head dims free
                for kc in range(4 * t4, 4 * (t4 + 1)):
                    ps = psp.tile([128, 1024], f32, tag="ps")
                    for c in range(8):
                        nc.tensor.matmul(
                            out=ps[:, 0:DL],
                            lhsT=xt[:, c, 128 * kc:128 * (kc + 1)],
                            rhs=wv_t[:, c, :],
                            start=(c == 0), stop=(c == 7),
                        )
                    pv = ps[:, 0:DL].rearrange("p (pair x) -> p pair x", pair=NP)
                    nc.vector.tensor_copy(out=vv[:, kc, :, 0:64],
                                          in_=pv[:, :, 0:64])
                    nc.vector.tensor_copy(out=vv[:, kc, :, 192:256],
                                          in_=pv[:, :, 64:128])

        # LN constants are not needed until late; keep their DMAs off the
        # startup critical path
        nc.sync.dma_start(out=lng_t, in_=lng_d.partition_broadcast(128))
        nc.sync.dma_start(out=lnb_t, in_=lnb_d.partition_broadcast(128))

        # ---------------- stage B (attention) + stage C (proj/RS/LN)
        with tc.tile_pool(name="ptp", bufs=4) as ptp, \
             tc.tile_pool(name="ohsp", bufs=2) as ohsp, \
             tc.tile_pool(name="recp", bufs=2) as recp, \
             tc.tile_pool(name="zevp", bufs=2) as zevp, \
             tc.tile_pool(name="lnp", bufs=2) as lnp:

            def norm_evac(oh):
                """Evacuate Oh+sums to SBUF right behind the last P@v."""
                ohs = ohsp.tile([128, 512], bf16, tag="ohs")
                nc.vector.tensor_copy(out=ohs, in_=oh)
                return ohs

            def norm_finish(qb, h, ohs):
                """Broadcast softmax sums, normalize into ohn. Deferred one
                head so the rb matmul never head-blocks the PE queue on the
                DVE evacuation."""
                pair = h // 2
                odd = h % 2
                rb = psp.tile([128, 1024], f32, tag="ps")
                rec = recp.tile([128, 512], f32, tag="rec")
                tsl = slice(QB * qb, QB * (qb + 1))
                # sums sit on row 64 (even head) / row 0 (odd head); the K=1
                # matmul outer-products that row across all 128 partitions
                srow = 0 if odd else 64
                dst = slice(64, 128) if odd else slice(0, 64)
                nc.tensor.matmul(out=rb[:, 0:512],
                                 lhsT=ones_t[srow:srow + 1, :],
                                 rhs=ohs[srow:srow + 1, :],
                                 start=True, stop=True)
                # full-tile: the custom-DVE reciprocal misbehaves at a
                # non-zero base partition; rb is identical on every row
                nc.vector.reciprocal_approx_fast(out=rec, in_=rb[:, 0:512])
                nc.vector.tensor_mul(out=ohn[dst, pair, tsl],
                                     in0=ohs[dst, :], in1=rec[dst, :])

            def oproj_tcl(qb, tcl):
                """One 128-token output-projection chunk; RS after the last."""
                t0 = QB * qb + 128 * tcl
                zp = psp.tile([128, 1024], f32, tag="ps")
                for ec in range(2):
                    for pair in range(NP):
                        nc.tensor.matmul(
                            out=zp[:, 512 * ec:512 * (ec + 1)],
                            lhsT=ohn[:, pair, t0:t0 + 128],
                            rhs=wot_t[:, pair, 512 * ec:512 * (ec + 1)],
                            start=(pair == 0), stop=(pair == NP - 1),
                        )
                zev = zevp.tile([128, D], bf16)
                nc.vector.tensor_copy(out=zev, in_=zp)
                nc.sync.dma_start(
                    out=ccin[qb][128 * tcl:128 * (tcl + 1), :], in_=zev)
                if tcl == QB // 128 - 1:
                    nc.gpsimd.collective_compute(
                        "ReduceScatter", Alu.add,
                        ins=[ccin[qb][:]], outs=[ccout[qb][:]],
                        replica_groups=RG,
                    )

            def layernorm(qb):
                """Residual + LayerNorm on this core's scattered shard."""
                ccz = lnp.tile([128, D], bf16, tag="ccz")
                nc.sync.dma_start(out=ccz, in_=ccout[qb])
                zt = lnp.tile([128, D], f32, tag="zt")
                nc.vector.tensor_copy(out=zt, in_=ccz)
                nc.vector.tensor_add(out=zt, in0=zt, in1=xres_t[:, qb, :])

                stats = lnp.tile([128, 2, 6], f32, tag="stats")
                for sg in range(2):
                    nc.vector.bn_stats(out=stats[:, sg, :],
                                       in_=zt[:, 512 * sg:512 * (sg + 1)])
                mv = lnp.tile([128, 2], f32, tag="mv")
                nc.vector.bn_aggr(out=mv, in_=stats)

                # rstd = rsqrt(var + eps), DVE-only (avoids ACT table thrash)
                ve = lnp.tile([128, 1], f32, tag="ve")
                nc.vector.tensor_scalar_add(out=ve, in0=mv[:, 1:2], scalar1=LN_EPS)
                y = lnp.tile([128, 1], f32, tag="y")
                nc.vector.tensor_scalar(
                    out=y.bitcast(i32), in0=ve.bitcast(i32), scalar1=1,
                    scalar2=None, op0=Alu.logical_shift_right)
                nc.vector.tensor_sub(out=y.bitcast(i32), in0=magic_t,
                                     in1=y.bitcast(i32))
                tnw = lnp.tile([128, 1], f32, tag="tnw")
                for _ in range(2):
                    nc.vector.tensor_mul(out=tnw, in0=ve, in1=y)
                    nc.vector.tensor_mul(out=tnw, in0=tnw, in1=y)
                    nc.vector.tensor_scalar(out=tnw, in0=tnw, scalar1=-0.5,
                                            scalar2=1.5, op0=Alu.mult, op1=Alu.add)
                    nc.vector.tensor_mul(out=y, in0=y, in1=tnw)

                nc.vector.tensor_scalar(out=zt, in0=zt, scalar1=mv[:, 0:1],
                                        scalar2=y, op0=Alu.subtract, op1=Alu.mult)
                nc.vector.tensor_mul(out=zt, in0=zt, in1=lng_t)
                nc.vector.tensor_add(out=zt, in0=zt, in1=lnb_t)
                nc.sync.dma_start(out=out_d[128 * qb:128 * (qb + 1), :], in_=zt)

            pending = None
            for qb in range(NQB):  # noqa: B007

                for h in range(HL):
                    mi = h // 2
                    ktp = kt_ev if h % 2 == 0 else kt_od
                    qT_b = qt[:, mi, QB * qb:QB * (qb + 1)]
                    oh = ohp.tile([128, 512], f32, tag="oh")
                    for g in range(8):
                        # spread the previous block's output projection over
                        # this head's score groups so its PSUM slots never
                        # bunch up in the rotation
                        if qb > 0 and h == 1 and g >= 4:
                            oproj_tcl(qb - 1, g - 4)
                        st = psp.tile([128, 1024], f32, tag="ps")
                        for i in range(2):
                            kc = 2 * g + i
                            nc.tensor.matmul(
                                out=st[:, 512 * i:512 * (i + 1)],
                                lhsT=ktp[:, mi, 128 * kc:128 * (kc + 1)],
                                rhs=qT_b,
                                start=True, stop=True,
                            )
                        pt = ptp.tile([128, 1024], bf16, tag="pt")
                        nc.scalar.activation(out=pt, in_=st, func=Exp)
                        for i in range(2):
                            kc = 2 * g + i
                            nc.tensor.matmul(
                                out=oh,
                                lhsT=vaug[:, kc, 128 * h:128 * (h + 1)],
                                rhs=pt[:, 512 * i:512 * (i + 1)],
                                start=(kc == 0), stop=(kc == NKC - 1),
                            )
                    ohs = norm_evac(oh)
                    if pending is not None:
                        norm_finish(*pending)
                    pending = (qb, h, ohs)
                if qb == NQB - 1:
                    norm_finish(*pending)
                    pending = None
                    for tcl in range(QB // 128):
                        oproj_tcl(qb, tcl)

            # LN blocks run after the whole attention DVE stream: their data
            # (ReduceScatter outputs) is long ready by then for qb 0..2, and
            # the DVE work overlaps the final ReduceScatter waits, so the
            # in-order DVE queue can never block attention work.
            for qb in range(NQB):
                with tc.tile_wait_until(LN_HINTS[qb]):
                    layernorm(qb)

    nc.compile()
    return nc


def _get_program():
    global _PROGRAM
    if _PROGRAM is None:
        _PROGRAM = _build_program()
    return _PROGRAM


def kernel(X, Y, Wq, Wk, Wv, cb, Wo_w, Wo_b, ln_g, ln_b):
    import ml_dtypes
    from concourse import bass_utils

    prog = _get_program()
    bf = ml_dtypes.bfloat16

    X = np.asarray(X, dtype=np.float32)
    Wq = np.asarray(Wq, dtype=np.float32)
    Wk = np.asarray(Wk, dtype=np.float32)
    Wv = np.asarray(Wv, dtype=np.float32)
    cb = np.asarray(cb, dtype=np.float32)
    Wo_w = np.asarray(Wo_w, dtype=np.float32)
    Wo_b = np.asarray(Wo_b, dtype=np.float32)
    ln_g = np.asarray(ln_g, dtype=np.float32)
    ln_b = np.asarray(ln_b, dtype=np.float32)

    WoT = np.ascontiguousarray(Wo_w.T)

    def core_rows(r):
        """Global row indices (within a batch) this core outputs, in order."""
        return np.concatenate(
            [np.arange(QB * t + 128 * r, QB * t + 128 * r + 128)
             for t in range(NQB)])

    in_maps = []
    for c in range(NCORES):
        b, hp, r = c // GROUP, c % GROUP, c % GROUP
        Xb = X[b]
        rows = core_rows(r)
        csl = slice(DL * hp, DL * (hp + 1))
        in_maps.append({
            "xt": np.ascontiguousarray(Xb.T).astype(bf),
            "xres": np.ascontiguousarray(Xb[rows] + Wo_b),
            "wq": np.ascontiguousarray(Wq[:, csl]).astype(bf),
            "wk": np.ascontiguousarray(Wk[:, csl]).astype(bf),
            "wv": np.ascontiguousarray(Wv[:, csl]).astype(bf),
            "wot": np.ascontiguousarray(WoT[csl, :]).astype(bf),
            "cb": np.ascontiguousarray(cb[csl].reshape(DL, 1)),
            "lng": np.ascontiguousarray(ln_g.reshape(1, D)),
            "lnb": np.ascontiguousarray(ln_b.reshape(1, D)),
        })

    res = bass_utils.run_bass_kernel_spmd(prog, in_maps, core_ids=list(range(NCORES)))
    global LAST_RESULT
    LAST_RESULT = res

    out = np.empty((B, L, D), np.float32)
    for cid in range(NCORES):
        b, r = cid // GROUP, cid % GROUP
        o = res.results[cid]["out"]
        out[b, core_rows(r)] = o
    return out


if __name__ == "__main__":
    rng = np.random.default_rng(0)
    ins = {
        "X": rng.standard_normal((B, L, D)).astype(np.float32),
        "Y": rng.standard_normal((B, L, D)).astype(np.float32),
        "Wq": (rng.uniform(-1, 1, (D, D)) / 32).astype(np.float32),
        "Wk": (rng.uniform(-1, 1, (D, D)) / 32).astype(np.float32),
        "Wv": (rng.uniform(-1, 1, (D, D)) / 32).astype(np.float32),
        "cb": np.zeros(D, np.float32),
        "Wo_w": (rng.uniform(-1, 1, (D, D)) / 32).astype(np.float32),
        "Wo_b": (rng.uniform(-1, 1, D) / 32).astype(np.float32),
        "ln_g": np.ones(D, np.float32),
        "ln_b": np.zeros(D, np.float32),
    }
    out = kernel(**ins)
    print("out", out.shape, out.dtype, float(np.abs(out).max()))
    print("exec_time_ns:", LAST_RESULT.exec_time_ns)


# revision 26
# speedup vs baseline: 1.0981x; 1.0811x over previous
"""Trainium2 Bass kernel for nn_Attention_90220083019846.

Multi-head attention block: q/k/v = X@W{q,k,v}, scores = (q+cb)@k^T,
softmax, O = P@v, Z = X + O@Wo^T + b, LayerNorm.

Sharding over 8 NeuronCores: data-parallel over batch (2 groups of 4 cores) x
tensor-parallel over heads (4 heads per core). Output projection partial sums
are combined with a ReduceScatter within each batch group; residual +
LayerNorm run on the scattered shards.

v2 rework, driven by the p-state ramp model (a stalled PE restarts at half
clock for 3us): the whole PE stream is kept gapless. Score PSUM rotates
through three 2-bank slots so the scheduler can run scores two groups ahead
of the exp; the P@v accumulator has two banks so the next head starts while
the previous head's Oh is evacuated. Head pairs are packed: the odd head's v
columns sit at 64..127 (ones column at 63), so Oh lands on partitions 64..127
and the output projection contracts a full K=128 with no zero padding,
halving its matmul count. The softmax-sums broadcast rides a full-rate f32r
matmul whose slot reuses the score pool. LayerNorm residuals (with Wo_b
pre-added on host) are preloaded to SBUF and the LN blocks are slotted into
the DVE stream behind each ReduceScatter via schedule hints; the last query
block reduce-scatters in 128-row chunks to shrink the tail.
"""

import contextlib
import ctypes
import sys
import types

sys.path.insert(0, "/opt/trn_rl_repo")

import numpy as np

# ---------------------------------------------------------------- profile hook
# The agent image's antenv lacks axon_hooks; provide it so that
# run_bass_kernel_spmd(trace=True) / BASS_TRACE=1 can capture NTFF profiles.
def _install_profile_hook():
    if "antenv.axon_hooks" in sys.modules:
        return
    try:
        import antenv
    except ImportError:
        return
    mod = types.ModuleType("antenv.axon_hooks")
    mod._hook = None
    mod.set_axon_ntff_profile_hook = lambda h: setattr(mod, "_hook", h)
    mod.get_axon_ntff_profile_hook = lambda: mod._hook
    sys.modules["antenv.axon_hooks"] = mod
    antenv.axon_hooks = mod
    try:
        lib = ctypes.CDLL("/opt/axon/libaxon_pjrt.so")
        if not hasattr(lib, "axon_start_nrt_profile"):
            return
        lib.axon_start_nrt_profile.argtypes = [
            ctypes.POINTER(ctypes.c_int64),
            ctypes.c_size_t,
        ]
        lib.axon_start_nrt_profile.restype = ctypes.c_int64
        lib.axon_stop_nrt_profile.argtypes = [ctypes.c_char_p]
        lib.axon_stop_nrt_profile.restype = ctypes.c_int64

        @contextlib.contextmanager
        def _hook(output_dir, device_ids):
            import jax

            jax.devices()
            if device_ids:
                ids = (ctypes.c_int64 * len(device_ids))(*device_ids)
                rc = lib.axon_start_nrt_profile(ids, len(device_ids))
            else:
                rc = lib.axon_start_nrt_profile(None, 0)
            if rc != 0:
                raise RuntimeError(f"axon_start_nrt_profile rc={rc}")
            try:
                yield
            finally:
                n = lib.axon_stop_nrt_profile(str(output_dir).encode())
                print(f"profile: {n} file(s) written to {output_dir}", file=sys.stderr)

        mod.set_axon_ntff_profile_hook(_hook)
    except OSError:
        pass


_install_profile_hook()

# ------------------------------------------------------------------- constants
B, L, D, H, HD = 2, 2048, 1024, 16, 64
NCORES = 8
GROUP = 4            # cores per batch group (tensor-parallel over heads)
HL = H // GROUP      # local heads per core
NP = HL // 2         # local head pairs
DL = HL * HD         # local head dims per core
QB = 512             # query block (tokens per pipeline chunk)
NQB = L // QB
NKC = L // 128       # key chunks
RG = [[0, 1, 2, 3], [4, 5, 6, 7]]
LN_EPS = 1e-5
RSQRT_MAGIC = 0x5F3759DF
# schedule hints (ms of estimated time) for when each LN block's DVE work may
# be slotted in; biased late so a tardy ReduceScatter never blocks the
# in-order DVE queue ahead of attention work
LN_HINTS = [0.200, 0.203, 0.206, 0.209]

_PROGRAM = None
LAST_RESULT = None


def _build_program():
    import concourse.tile as tile
    from concourse import bacc, mybir

    fr = mybir.dt.float32r
    f32 = mybir.dt.float32
    bf16 = mybir.dt.bfloat16
    i32 = mybir.dt.int32
    Exp = mybir.ActivationFunctionType.Exp
    Alu = mybir.AluOpType

    nc = bacc.Bacc("TRN2", target_bir_lowering=False, debug=False,
                   num_devices=NCORES)

    xt_d = nc.dram_tensor("xt", (D, L), bf16, kind="ExternalInput").ap()
    wq_d = nc.dram_tensor("wq", (D, DL), bf16, kind="ExternalInput").ap()
    wk_d = nc.dram_tensor("wk", (D, DL), bf16, kind="ExternalInput").ap()
    wv_d = nc.dram_tensor("wv", (D, DL), bf16, kind="ExternalInput").ap()
    wot_d = nc.dram_tensor("wot", (DL, D), bf16, kind="ExternalInput").ap()
    cb_d = nc.dram_tensor("cb", (DL, 1), f32, kind="ExternalInput").ap()
    xres_d = nc.dram_tensor("xres", (QB, D), f32, kind="ExternalInput").ap()
    lng_d = nc.dram_tensor("lng", (1, D), f32, kind="ExternalInput").ap()
    lnb_d = nc.dram_tensor("lnb", (1, D), f32, kind="ExternalInput").ap()
    out_d = nc.dram_tensor("out", (QB, D), f32, kind="ExternalOutput").ap()

    ccin = [nc.dram_tensor(f"ccin{t}", (QB, D), bf16, kind="Internal").ap()
            for t in range(NQB)]
    ccout = [nc.dram_tensor(f"ccout{t}", (QB // GROUP, D), bf16,
                            kind="Internal").ap()
             for t in range(NQB)]

    with tile.TileContext(nc) as tc, contextlib.ExitStack() as ctx:
        # ---------------- persistent pools
        wp = ctx.enter_context(tc.tile_pool(name="wp", bufs=1))
        kqv = ctx.enter_context(tc.tile_pool(name="kqv", bufs=1))
        cons = ctx.enter_context(tc.tile_pool(name="cons", bufs=1))
        # PSUM: 3 score slots x 2 banks + 2 P@v accumulator banks = 8 banks.
        # The sums-broadcast (rb) and output-projection (zp) tiles rotate
        # through the score pool.
        psp = ctx.enter_context(tc.tile_pool(name="psp", bufs=3, space="PSUM"))
        ohp = ctx.enter_context(tc.tile_pool(name="ohp", bufs=2, space="PSUM"))

        wq_t = wp.tile([128, 8, DL], bf16)
        wk_t = wp.tile([128, 8, DL], bf16)
        wv_t = wp.tile([128, 8, DL], bf16)
        wot_t = wp.tile([128, NP, D], bf16)
        # wk lands first, split per contraction chunk so the very first
        # projection matmul only waits on chunk 0
        wk_r = wk_d.rearrange("(c p) o -> p c o", p=128)
        for c in range(8):
            nc.sync.dma_start(out=wk_t[:, c, :], in_=wk_r[:, c, :])

        # k^T with the other head of the pair zeroed (full-square lhsT);
        # q^T keeps both heads (zero weights ignore the other head's rows)
        kt_ev = kqv.tile([128, 2, L], bf16)
        kt_od = kqv.tile([128, 2, L], bf16)
        qt = kqv.tile([128, 2, L], bf16)     # q^T (+cb)
        # v | ones columns; per head pair: even head dims at block cols 0..63
        # of its 128-block with ones at 64, odd head dims at 64..127 of its
        # block with ones at 0
        vaug = kqv.tile([128, NKC, HL * 128], bf16)
        ohn = kqv.tile([128, NP, L], bf16)   # normalized Oh^T, head pairs packed
        xres_t = kqv.tile([128, 4, D], f32)  # residual rows (+Wo_b), preloaded

        cb_t = cons.tile([128, 2], f32)
        nc.sync.dma_start(out=cb_t, in_=cb_d.rearrange("(m p) x -> p (m x)", p=128))
        # lhsT for the K=1 sums-broadcast outer product (any single row).
        # bf16: the f32r matmul path only yields valid output on the first
        # 64 partitions, bf16 covers all 128.
        ones_t = cons.tile([128, 128], bf16)
        nc.vector.memset(ones_t, 1.0)
        lng_t = cons.tile([128, D], f32)
        lnb_t = cons.tile([128, D], f32)
        magic_t = cons.tile([128, 1], i32)
        nc.vector.memset(magic_t, RSQRT_MAGIC)
        # row masks: keep one head of a pair, zero the other (f32 scalars)
        mask_lo = cons.tile([128, 1], f32)
        mask_hi = cons.tile([128, 1], f32)
        nc.vector.memset(mask_lo, 0.0)
        nc.vector.memset(mask_lo[0:64, :], 1.0)
        nc.vector.memset(mask_hi, 0.0)
        nc.vector.memset(mask_hi[64:128, :], 1.0)

        # vaug constant columns: ones at even-block col 64 / odd-block col 0
        # (pair-relative 128), zeros on the unused strips (the zeros feed
        # matmul contractions, so they must be real zeros, not garbage)
        vv = vaug.rearrange("p k (pair x) -> p k pair x", pair=NP)
        nc.vector.memset(vv[:, :, :, 64:65], 1.0)
        nc.vector.memset(vv[:, :, :, 128:129], 1.0)
        nc.vector.memset(vv[:, :, :, 65:128], 0.0)
        nc.vector.memset(vv[:, :, :, 129:192], 0.0)

        # ---------------- stage A: projections (needs X^T)
        with tc.tile_pool(name="xtp", bufs=1) as xtp:
            xt = xtp.tile([128, 8, L], bf16)
            for t4 in range(NQB):
                for c in range(8):
                    nc.sync.dma_start(
                        out=xt[:, c, QB * t4:QB * (t4 + 1)],
                        in_=xt_d[128 * c:128 * (c + 1), QB * t4:QB * (t4 + 1)])
                if t4 == 0:
                    nc.sync.dma_start(
                        out=wv_t, in_=wv_d.rearrange("(c p) o -> p c o", p=128))
                    nc.sync.dma_start(
                        out=wq_t, in_=wq_d.rearrange("(c p) o -> p c o", p=128))
                    nc.sync.dma_start(
                        out=wot_t,
                        in_=wot_d.rearrange("(h p) e -> p h e", p=128))
            nc.sync.dma_start(
                out=xres_t, in_=xres_d.rearrange("(q p) e -> p q e", p=128))

            # interleave k/q/v projections by t4 so compute starts as soon as
            # the first X chunk lands
            for t4 in range(NQB):
                tsl = slice(QB * t4, QB * (t4 + 1))
                for w_t, is_q in ((wk_t, False), (wq_t, True)):
                    for m in range(2):
                        ps = psp.tile([128, 1024], f32, tag="ps")
                        for c in range(8):
                            nc.tensor.matmul(
                                out=ps[:, 0:512],
                                lhsT=w_t[:, c, 128 * m:128 * (m + 1)],
                                rhs=xt[:, c, tsl],
                                start=(c == 0), stop=(c == 7),
                            )
                        if is_q:
                            # ACT is idle until the first exp; offload the
                            # bias-add evacuation there
                            nc.scalar.add(out=qt[:, m, tsl],
                                          in_=ps[:, 0:512],
                                          add=cb_t[:, m:m + 1])
                        else:
                            nc.vector.tensor_scalar_mul(
                                out=kt_ev[:, m, tsl], in0=ps[:, 0:512],
                                scalar1=mask_lo)
                            nc.scalar.activation(
                                out=kt_od[:, m, tsl], in_=ps[:, 0:512],
                                func=mybir.ActivationFunctionType.Identity,
                                bias=0.0, scale=mask_hi)
                # v: tokens on partitions, head dims free
                for kc in range(4 * t4, 4 * (t4 + 1)):
                    ps = psp.tile([128, 1024], f32, tag="ps")
                    for c in range(8):
                        nc.tensor.matmul(
                            out=ps[:, 0:DL],
                            lhsT=xt[:, c, 128 * kc:128 * (kc + 1)],
                            rhs=wv_t[:, c, :],
                            start=(c == 0), stop=(c == 7),
                        )
                    pv = ps[:, 0:DL].rearrange("p (pair x) -> p pair x", pair=NP)
                    nc.vector.tensor_copy(out=vv[:, kc, :, 0:64],
                                          in_=pv[:, :, 0:64])
                    nc.vector.tensor_copy(out=vv[:, kc, :, 192:256],
                                          in_=pv[:, :, 64:128])

        # LN constants are not needed until late; keep their DMAs off the
        # startup critical path
        nc.sync.dma_start(out=lng_t, in_=lng_d.partition_broadcast(128))
        nc.sync.dma_start(out=lnb_t, in_=lnb_d.partition_broadcast(128))

        # ---------------- stage B (attention) + stage C (proj/RS/LN)
        with tc.tile_pool(name="ptp", bufs=4) as ptp, \
             tc.tile_pool(name="ohsp", bufs=2) as ohsp, \
             tc.tile_pool(name="recp", bufs=2) as recp, \
             tc.tile_pool(name="zevp", bufs=2) as zevp, \
             tc.tile_pool(name="lnp", bufs=2) as lnp:

            def norm_evac(oh):
                """Evacuate Oh+sums to SBUF right behind the last P@v."""
                ohs = ohsp.tile([128, 512], bf16, tag="ohs")
                nc.vector.tensor_copy(out=ohs, in_=oh)
                return ohs

            def norm_finish(qb, h, ohs):
                """Broadcast softmax sums, normalize into ohn. Deferred one
                head so the rb matmul never head-blocks the PE queue on the
                DVE evacuation."""
                pair = h // 2
                odd = h % 2
                rb = psp.tile([128, 1024], f32, tag="ps")
                rec = recp.tile([128, 512], f32, tag="rec")
                tsl = slice(QB * qb, QB * (qb + 1))
                # sums sit on row 64 (even head) / row 0 (odd head); the K=1
                # matmul outer-products that row across all 128 partitions
                srow = 0 if odd else 64
                dst = slice(64, 128) if odd else slice(0, 64)
                nc.tensor.matmul(out=rb[:, 0:512],
                                 lhsT=ones_t[srow:srow + 1, :],
                                 rhs=ohs[srow:srow + 1, :],
                                 start=True, stop=True)
                # full-tile: the custom-DVE reciprocal misbehaves at a
                # non-zero base partition; rb is identical on every row
                nc.vector.reciprocal_approx_fast(out=rec, in_=rb[:, 0:512])
                nc.vector.tensor_mul(out=ohn[dst, pair, tsl],
                                     in0=ohs[dst, :], in1=rec[dst, :])

            def oproj_tcl(qb, tcl):
                """One 128-token output-projection chunk; RS after the last."""
                t0 = QB * qb + 128 * tcl
                zp = psp.tile([128, 1024], f32, tag="ps")
                for ec in range(2):
                    for pair in range(NP):
                        nc.tensor.matmul(
                            out=zp[:, 512 * ec:512 * (ec + 1)],
                            lhsT=ohn[:, pair, t0:t0 + 128],
                            rhs=wot_t[:, pair, 512 * ec:512 * (ec + 1)],
                            start=(pair == 0), stop=(pair == NP - 1),
                        )
                zev = zevp.tile([128, D], bf16)
                nc.vector.tensor_copy(out=zev, in_=zp)
                nc.sync.dma_start(
                    out=ccin[qb][128 * tcl:128 * (tcl + 1), :], in_=zev)
                if tcl == QB // 128 - 1:
                    nc.gpsimd.collective_compute(
                        "ReduceScatter", Alu.add,
                        ins=[ccin[qb][:]], outs=[ccout[qb][:]],
                        replica_groups=RG,
                    )

            def layernorm(qb):
                """Residual + LayerNorm on this core's scattered shard."""
                ccz = lnp.tile([128, D], bf16, tag="ccz")
                nc.sync.dma_start(out=ccz, in_=ccout[qb])
                zt = lnp.tile([128, D], f32, tag="zt")
                nc.vector.tensor_copy(out=zt, in_=ccz)
                nc.vector.tensor_add(out=zt, in0=zt, in1=xres_t[:, qb, :])

                stats = lnp.tile([128, 2, 6], f32, tag="stats")
                for sg in range(2):
                    nc.vector.bn_stats(out=stats[:, sg, :],
                                       in_=zt[:, 512 * sg:512 * (sg + 1)])
                mv = lnp.tile([128, 2], f32, tag="mv")
                nc.vector.bn_aggr(out=mv, in_=stats)

                # rstd = rsqrt(var + eps), DVE-only (avoids ACT table thrash)
                ve = lnp.tile([128, 1], f32, tag="ve")
                nc.vector.tensor_scalar_add(out=ve, in0=mv[:, 1:2], scalar1=LN_EPS)
                y = lnp.tile([128, 1], f32, tag="y")
                nc.vector.tensor_scalar(
                    out=y.bitcast(i32), in0=ve.bitcast(i32), scalar1=1,
                    scalar2=None, op0=Alu.logical_shift_right)
                nc.vector.tensor_sub(out=y.bitcast(i32), in0=magic_t,
                                     in1=y.bitcast(i32))
                tnw = lnp.tile([128, 1], f32, tag="tnw")
                for _ in range(2):
                    nc.vector.tensor_mul(out=tnw, in0=ve, in1=y)
                    nc.vector.tensor_mul(out=tnw, in0=tnw, in1=y)
                    nc.vector.tensor_scalar(out=tnw, in0=tnw, scalar1=-0.5,
                                            scalar2=1.5, op0=Alu.mult, op1=Alu.add)
                    nc.vector.tensor_mul(out=y, in0=y, in1=tnw)

                nc.vector.tensor_scalar(out=zt, in0=zt, scalar1=mv[:, 0:1],
                                        scalar2=y, op0=Alu.subtract, op1=Alu.mult)
                nc.vector.tensor_mul(out=zt, in0=zt, in1=lng_t)
                nc.vector.tensor_add(out=zt, in0=zt, in1=lnb_t)
                nc.sync.dma_start(out=out_d[128 * qb:128 * (qb + 1), :], in_=zt)

            pending = None
            for qb in range(NQB):  # noqa: B007

                for h in range(HL):
                    mi = h // 2
                    ktp = kt_ev if h % 2 == 0 else kt_od
                    qT_b = qt[:, mi, QB * qb:QB * (qb + 1)]
                    oh = ohp.tile([128, 512], f32, tag="oh")
                    for g in range(8):
                        st = psp.tile([128, 1024], f32, tag="ps")
                        for i in range(2):
                            kc = 2 * g + i
                            nc.tensor.matmul(
                                out=st[:, 512 * i:512 * (i + 1)],
                                lhsT=ktp[:, mi, 128 * kc:128 * (kc + 1)],
                                rhs=qT_b,
                                start=True, stop=True,
                            )
                        pt = ptp.tile([128, 1024], bf16, tag="pt")
                        nc.scalar.activation(out=pt, in_=st, func=Exp)
                        for i in range(2):
                            kc = 2 * g + i
                            nc.tensor.matmul(
                                out=oh,
                                lhsT=vaug[:, kc, 128 * h:128 * (h + 1)],
                                rhs=pt[:, 512 * i:512 * (i + 1)],
                                start=(kc == 0), stop=(kc == NKC - 1),
                            )
                    ohs = norm_evac(oh)
                    if pending is not None:
                        norm_finish(*pending)
                    pending = (qb, h, ohs)
                    # interleave the previous block's output projection midway
                    # through this block's attention
                    if qb > 0 and h == 1:
                        for tcl in range(QB // 128):
                            oproj_tcl(qb - 1, tcl)
                if qb == NQB - 1:
                    norm_finish(*pending)
                    pending = None
                    for tcl in range(QB // 128):
                        oproj_tcl(qb, tcl)

            # LN blocks run after the whole attention DVE stream: their data
            # (ReduceScatter outputs) is long ready by then for qb 0..2, and
            # the DVE work overlaps the final ReduceScatter waits, so the
            # in-order DVE queue can never block attention work.
            for qb in range(NQB):
                with tc.tile_wait_until(LN_HINTS[qb]):
                    layernorm(qb)

    nc.compile()
    return nc


def _get_program():
    global _PROGRAM
    if _PROGRAM is None:
        _PROGRAM = _build_program()
    return _PROGRAM


def kernel(X, Y, Wq, Wk, Wv, cb, Wo_w, Wo_b, ln_g, ln_b):
    import ml_dtypes
    from concourse import bass_utils

    prog = _get_program()
    bf = ml_dtypes.bfloat16

    X = np.asarray(X, dtype=np.float32)
    Wq = np.asarray(Wq, dtype=np.float32)
    Wk = np.asarray(Wk, dtype=np.float32)
    Wv = np.asarray(Wv, dtype=np.float32)
    cb = np.asarray(cb, dtype=np.float32)
    Wo_w = np.asarray(Wo_w, dtype=np.float32)
    Wo_b = np.asarray(Wo_b, dtype=np.float32)
    ln_g = np.asarray(ln_g, dtype=np.float32)
    ln_b = np.asarray(ln_b, dtype=np.float32)

    WoT = np.ascontiguousarray(Wo_w.T)

    def core_rows(r):
        """Global row indices (within a batch) this core outputs, in order."""
        return np.concatenate(
            [np.arange(QB * t + 128 * r, QB * t + 128 * r + 128)
             for t in range(NQB)])

    in_maps = []
    for c in range(NCORES):
        b, hp, r = c // GROUP, c % GROUP, c % GROUP
        Xb = X[b]
        rows = core_rows(r)
        csl = slice(DL * hp, DL * (hp + 1))
        in_maps.append({
            "xt": np.ascontiguousarray(Xb.T).astype(bf),
            "xres": np.ascontiguousarray(Xb[rows] + Wo_b),
            "wq": np.ascontiguousarray(Wq[:, csl]).astype(bf),
            "wk": np.ascontiguousarray(Wk[:, csl]).astype(bf),
            "wv": np.ascontiguousarray(Wv[:, csl]).astype(bf),
            "wot": np.ascontiguousarray(WoT[csl, :]).astype(bf),
            "cb": np.ascontiguousarray(cb[csl].reshape(DL, 1)),
            "lng": np.ascontiguousarray(ln_g.reshape(1, D)),
            "lnb": np.ascontiguousarray(ln_b.reshape(1, D)),
        })

    res = bass_utils.run_bass_kernel_spmd(prog, in_maps, core_ids=list(range(NCORES)))
    global LAST_RESULT
    LAST_RESULT = res

    out = np.empty((B, L, D), np.float32)
    for cid in range(NCORES):
        b, r = cid // GROUP, cid % GROUP
        o = res.results[cid]["out"]
        out[b, core_rows(r)] = o
    return out


if __name__ == "__main__":
    rng = np.random.default_rng(0)
    ins = {
        "X": rng.standard_normal((B, L, D)).astype(np.float32),
        "Y": rng.standard_normal((B, L, D)).astype(np.float32),
        "Wq": (rng.uniform(-1, 1, (D, D)) / 32).astype(np.float32),
        "Wk": (rng.uniform(-1, 1, (D, D)) / 32).astype(np.float32),
        "Wv": (rng.uniform(-1, 1, (D, D)) / 32).astype(np.float32),
        "cb": np.zeros(D, np.float32),
        "Wo_w": (rng.uniform(-1, 1, (D, D)) / 32).astype(np.float32),
        "Wo_b": (rng.uniform(-1, 1, D) / 32).astype(np.float32),
        "ln_g": np.ones(D, np.float32),
        "ln_b": np.zeros(D, np.float32),
    }
    out = kernel(**ins)
    print("out", out.shape, out.dtype, float(np.abs(out).max()))
    print("exec_time_ns:", LAST_RESULT.exec_time_ns)
